# revision 2
# baseline (speedup 1.0000x reference)
"""Trainium2 Bass kernel for fused multi-tensor cosine-similarity loss.

Computes 1 - <r,d> / (|r| |d|) over 10 gradient tensors (5 rec + 5 data,
45,675,264 f32 elements per side), data-parallel across 8 NeuronCores.

Strategy (memory-bound, ~45.7 MB HBM traffic per core, per-core DMA cap
~435 GB/s, paired cores share ~716 GB/s):
  - Host packs each side into a flat f32 stream, zero-padded to
    8 cores x 44,605 columns x 128 partitions; per core that is five
    contiguous [128, 8192] tiles plus 2048/1024/573-column tail chunks
    (descending so the final DMA's compute drain is minimal).
  - Dual-ring DMA: the rec stream is issued on the SP (sync) HWDGE ring,
    the data stream on the Activation ring.  Two concurrent request
    streams keep more DMAs outstanding, which wins a fair share of the
    paired-core HBM arbitration (single-ring cores can get starved to
    ~300 GB/s when their partner core streams at 420).
  - Few, large DMAs (8 per side vs 44 total in v1) cut the semaphore
    count, which directly shrinks the Tile epilogue (each allocated
    semaphore costs an individual clear instruction at exit).
  - Compute in [128, 2048] slices: DVE does the dot (r*d with row-sum
    accum) on every slice plus r^2 on odd slices; ACT does d^2 on every
    slice plus r^2 on even slices.  Engine scratch outs go to PSUM
    (4 banks each), freeing SBUF entirely for DMA double-buffering.
  - All per-slice partial sums land as columns of one [128, 69]
    accumulator, DMA'd out once; host reduces in float64 and applies
    the final cosine combine.
"""

import sys

import numpy as np

_REPO = "/opt/trn_rl_repo"
if _REPO not in sys.path:
    sys.path.insert(0, _REPO)

import concourse.bacc as bacc
import concourse.mybir as mybir
from concourse.bass_utils import run_bass_kernel_spmd
from concourse.tile import TileContext

C = 8  # cores
P = 128  # SBUF partitions
FD = 8192  # columns per full DMA tile (4 MiB)
FC = 2048  # columns per compute slice (PSUM-scratch sized)
TOTAL = 45_675_264  # elements per side (sum of the 5 tensor sizes)
PER_CORE = TOTAL // C  # 5,709,408
COLS = -(-PER_CORE // P)  # 44,605 columns per core (32 pad elems)
NFULL = COLS // FD  # 5 full [P, FD] tiles
TAILS = [2048, 1024, COLS - NFULL * FD - 3072]  # 2048 + 1024 + 573
assert NFULL * FD + sum(TAILS) == COLS
PADDED_PER_CORE = P * COLS  # 5,709,440

# DMA tile list: (kind, index_or_tag, ncols). Full tiles first, then
# descending tail chunks so the last-arriving data needs the least compute.
_DMA_TILES = [("f", i, FD) for i in range(NFULL)] + [
    ("t", j, w) for j, w in enumerate(TAILS)
]
# compute slices per DMA tile: list of (col_offset_in_tile, width, global_slice)
_SLICES = {}
_g = 0
for kind, idx, w in _DMA_TILES:
    sl = []
    off = 0
    while off < w:
        cw = min(FC, w - off)
        sl.append((off, cw, _g))
        _g += 1
        off += cw
    _SLICES[(kind, idx)] = sl
NSLICE = _g  # 23
ACC_W = 3 * NSLICE  # dot | rr | dd column blocks

_REC_KEYS = ("rec_emb", "rec_qkv", "rec_proj", "rec_fc1", "rec_fc2")
_DATA_KEYS = ("data_emb", "data_qkv", "data_proj", "data_fc1", "data_fc2")

_CACHE = {}


def _build():
    nc = bacc.Bacc("TRN2", target_bir_lowering=False, debug=False)
    f32 = mybir.dt.float32
    r0 = nc.declare_dram_parameter("r0", [NFULL, P, FD], f32, isOutput=False)
    d0 = nc.declare_dram_parameter("d0", [NFULL, P, FD], f32, isOutput=False)
    rt = [
        nc.declare_dram_parameter(f"rt{j}", [P, w], f32, isOutput=False)
        for j, w in enumerate(TAILS)
    ]
    dt = [
        nc.declare_dram_parameter(f"dt{j}", [P, w], f32, isOutput=False)
        for j, w in enumerate(TAILS)
    ]
    o = nc.declare_dram_parameter("o", [P, ACC_W], f32, isOutput=True)

    def src(side, kind, idx):
        if kind == "f":
            return (r0 if side == "r" else d0)[idx]
        return (rt if side == "r" else dt)[idx][:]

    with TileContext(nc) as tc:
        with (
            tc.tile_pool(name="io", bufs=2) as io,
            tc.tile_pool(name="accp", bufs=1) as accp,
            tc.psum_pool(name="scr", bufs=1) as scr,
        ):
            acc = accp.tile([P, ACC_W], f32)
            v_scr = scr.tile([P, FC], f32, tag="v")  # DVE scratch, 4 banks
            a_scr = scr.tile([P, FC], f32, tag="a")  # ACT scratch, 4 banks

            def compute(tiles):
                rtile, dtile, key = tiles
                for off, cw, g in _SLICES[key]:
                    rs = rtile[:, off : off + cw]
                    ds = dtile[:, off : off + cw]
                    # dot: out = (rs bypass 1.0) * ds; accum_out = row-sum
                    nc.vector.scalar_tensor_tensor(
                        out=v_scr[:, :cw],
                        in0=rs,
                        scalar=1.0,
                        in1=ds,
                        op0=mybir.AluOpType.bypass,
                        op1=mybir.AluOpType.mult,
                        accum_out=acc[:, g : g + 1],
                    )
                    # rr: alternate engines so DVE/ACT stay balanced
                    if g % 2 == 1:
                        nc.vector.scalar_tensor_tensor(
                            out=v_scr[:, :cw],
                            in0=rs,
                            scalar=1.0,
                            in1=rs,
                            op0=mybir.AluOpType.bypass,
                            op1=mybir.AluOpType.mult,
                            accum_out=acc[:, NSLICE + g : NSLICE + g + 1],
                        )
                    else:
                        nc.scalar.activation(
                            a_scr[:, :cw],
                            rs,
                            mybir.ActivationFunctionType.Square,
                            accum_out=acc[:, NSLICE + g : NSLICE + g + 1],
                        )
                    # dd: always ACT
                    nc.scalar.activation(
                        a_scr[:, :cw],
                        ds,
                        mybir.ActivationFunctionType.Square,
                        accum_out=acc[:, 2 * NSLICE + g : 2 * NSLICE + g + 1],
                    )

            prev = None
            for kind, idx, w in _DMA_TILES:
                rtile = io.tile([P, w], f32, tag=f"r{kind}{idx if kind=='t' else ''}",
                                padded_shape=[P, FD] if kind == "f" else None,
                                bufs=2 if kind == "f" else 1)
                dtile = io.tile([P, w], f32, tag=f"d{kind}{idx if kind=='t' else ''}",
                                padded_shape=[P, FD] if kind == "f" else None,
                                bufs=2 if kind == "f" else 1)
                # rec stream on the SP ring, data stream on the ACT ring
                nc.sync.dma_start(out=rtile[:], in_=src("r", kind, idx))
                nc.scalar.dma_start(out=dtile[:], in_=src("d", kind, idx))
                if prev is not None:
                    compute(prev)
                prev = (rtile, dtile, (kind, idx))
            compute(prev)
            nc.sync.dma_start(out=o[:], in_=acc[:])
    nc.compile()
    return nc


def _get_nc():
    if "nc" not in _CACHE:
        _CACHE["nc"] = _build()
    return _CACHE["nc"]


def _pack(arrays):
    flat = np.concatenate([np.asarray(a, dtype=np.float32).reshape(-1) for a in arrays])
    assert flat.size == TOTAL
    buf = np.zeros((C, PADDED_PER_CORE), dtype=np.float32)
    for c in range(C):
        buf[c, :PER_CORE] = flat[c * PER_CORE : (c + 1) * PER_CORE]
    nmain = NFULL * P * FD
    main = buf[:, :nmain].reshape(C, NFULL, P, FD)
    tails = []
    off = nmain
    for w in TAILS:
        tails.append(buf[:, off : off + P * w].reshape(C, P, w))
        off += P * w
    return main, tails


def _run(inputs, trace=False, trace_cores=None):
    rmain, rtails = _pack([inputs[k] for k in _REC_KEYS])
    dmain, dtails = _pack([inputs[k] for k in _DATA_KEYS])
    in_maps = []
    for c in range(C):
        m = {"r0": rmain[c], "d0": dmain[c]}
        for j in range(len(TAILS)):
            m[f"rt{j}"] = rtails[j][c]
            m[f"dt{j}"] = dtails[j][c]
        in_maps.append(m)
    kwargs = {}
    if trace_cores is not None:
        kwargs["trace_cores"] = trace_cores
    res = run_bass_kernel_spmd(
        _get_nc(), in_maps, core_ids=list(range(C)), trace=trace, **kwargs
    )
    tot = np.zeros(3, dtype=np.float64)
    for m in res.results:
        a = m["o"].astype(np.float64)
        tot[0] += a[:, :NSLICE].sum()
        tot[1] += a[:, NSLICE : 2 * NSLICE].sum()
        tot[2] += a[:, 2 * NSLICE :].sum()
    sp, rn, dn = tot
    out = 1.0 - sp / (np.sqrt(rn) * np.sqrt(dn))
    return np.array(out, dtype=np.float32), res


def kernel(**inputs):
    out, _ = _run(inputs, trace=False)
    return out


def kernel_traced(**inputs):
    out, res = _run(inputs, trace=True, trace_cores=list(range(C)))
    return out, res


# revision 5
# speedup vs baseline: 1.1245x; 1.1245x over previous
"""Trainium2 Bass kernel for fused multi-tensor cosine-similarity loss.

Computes 1 - <r,d> / (|r| |d|) over 10 gradient tensors (5 rec + 5 data,
45,675,264 f32 elements per side), data-parallel across 8 NeuronCores.

Strategy (memory-bound, ~45.7 MB HBM traffic per core, per-core DMA cap
~435 GB/s, paired cores share ~716 GB/s):
  - Host packs each side into a flat f32 stream, zero-padded to
    8 cores x 44,605 columns x 128 partitions; per core that is five
    contiguous [128, 8192] tiles plus 2048/1024/573-column tail chunks
    (descending so the final DMA's compute drain is minimal).
  - Dual-ring DMA: the rec stream is issued on the SP (sync) HWDGE ring,
    the data stream on the Activation ring.  Two concurrent request
    streams keep more DMAs outstanding, which wins a fair share of the
    paired-core HBM arbitration (single-ring cores can get starved to
    ~300 GB/s when their partner core streams at 420).
  - Few, large DMAs (8 per side vs 44 total in v1) cut the semaphore
    count, which directly shrinks the Tile epilogue (each allocated
    semaphore costs an individual clear instruction at exit).
  - Compute in [128, 2048] slices: DVE does the dot (r*d with row-sum
    accum) on every slice plus r^2 on odd slices; ACT does d^2 on every
    slice plus r^2 on even slices.  Engine scratch outs go to PSUM
    (4 banks each), freeing SBUF entirely for DMA double-buffering.
  - All per-slice partial sums land as columns of one [128, 69]
    accumulator, DMA'd out once; host reduces in float64 and applies
    the final cosine combine.
"""

import sys

import numpy as np

_REPO = "/opt/trn_rl_repo"
if _REPO not in sys.path:
    sys.path.insert(0, _REPO)

import concourse.bacc as bacc
import concourse.mybir as mybir
from concourse.bass_utils import run_bass_kernel_spmd
from concourse.tile import TileContext

C = 8  # cores
P = 128  # SBUF partitions
FD = 4096  # columns per full DMA tile (2 MiB)
FC = 2048  # columns per compute slice (PSUM-scratch sized)
TOTAL = 45_675_264  # elements per side (sum of the 5 tensor sizes)
PER_CORE = TOTAL // C  # 5,709,408
COLS = -(-PER_CORE // P)  # 44,605 columns per core (32 pad elems)
NFULL = COLS // FD  # 10 full [P, FD] tiles
TAILS = [2048, 1024, COLS - NFULL * FD - 3072]  # 2048 + 1024 + 573
assert NFULL * FD + sum(TAILS) == COLS
PADDED_PER_CORE = P * COLS  # 5,709,440

# DMA tile list: (kind, index_or_tag, ncols). Full tiles first, then
# descending tail chunks so the last-arriving data needs the least compute.
_DMA_TILES = [("f", i, FD) for i in range(NFULL)] + [
    ("t", j, w) for j, w in enumerate(TAILS)
]
# compute slices per DMA tile: list of (col_offset_in_tile, width, global_slice)
_SLICES = {}
_g = 0
for kind, idx, w in _DMA_TILES:
    sl = []
    off = 0
    while off < w:
        cw = min(FC, w - off)
        sl.append((off, cw, _g))
        _g += 1
        off += cw
    _SLICES[(kind, idx)] = sl
NSLICE = _g  # 23
ACC_W = 3 * NSLICE  # dot | rr | dd column blocks

_REC_KEYS = ("rec_emb", "rec_qkv", "rec_proj", "rec_fc1", "rec_fc2")
_DATA_KEYS = ("data_emb", "data_qkv", "data_proj", "data_fc1", "data_fc2")

_CACHE = {}


def _build():
    nc = bacc.Bacc("TRN2", target_bir_lowering=False, debug=False)
    f32 = mybir.dt.float32
    r0 = nc.declare_dram_parameter("r0", [NFULL, P, FD], f32, isOutput=False)
    d0 = nc.declare_dram_parameter("d0", [NFULL, P, FD], f32, isOutput=False)
    rt = [
        nc.declare_dram_parameter(f"rt{j}", [P, w], f32, isOutput=False)
        for j, w in enumerate(TAILS)
    ]
    dt = [
        nc.declare_dram_parameter(f"dt{j}", [P, w], f32, isOutput=False)
        for j, w in enumerate(TAILS)
    ]
    o = nc.declare_dram_parameter("o", [P, ACC_W], f32, isOutput=True)

    def src(side, kind, idx):
        if kind == "f":
            return (r0 if side == "r" else d0)[idx]
        return (rt if side == "r" else dt)[idx][:]

    with TileContext(nc) as tc:
        with (
            tc.tile_pool(name="io", bufs=2) as io,
            tc.tile_pool(name="accp", bufs=1) as accp,
            tc.psum_pool(name="scr", bufs=1) as scr,
        ):
            acc = accp.tile([P, ACC_W], f32)
            v_scr = scr.tile([P, FC], f32, tag="v")  # DVE scratch, 4 banks
            a_scr = scr.tile([P, FC], f32, tag="a")  # ACT scratch, 4 banks

            def compute(tiles):
                rtile, dtile, key = tiles
                for off, cw, g in _SLICES[key]:
                    rs = rtile[:, off : off + cw]
                    ds = dtile[:, off : off + cw]
                    # dot: out = (rs bypass 1.0) * ds; accum_out = row-sum
                    nc.vector.scalar_tensor_tensor(
                        out=v_scr[:, :cw],
                        in0=rs,
                        scalar=1.0,
                        in1=ds,
                        op0=mybir.AluOpType.bypass,
                        op1=mybir.AluOpType.mult,
                        accum_out=acc[:, g : g + 1],
                    )
                    # rr: alternate engines so DVE/ACT stay balanced
                    if g % 2 == 1:
                        nc.vector.scalar_tensor_tensor(
                            out=v_scr[:, :cw],
                            in0=rs,
                            scalar=1.0,
                            in1=rs,
                            op0=mybir.AluOpType.bypass,
                            op1=mybir.AluOpType.mult,
                            accum_out=acc[:, NSLICE + g : NSLICE + g + 1],
                        )
                    else:
                        nc.scalar.activation(
                            a_scr[:, :cw],
                            rs,
                            mybir.ActivationFunctionType.Square,
                            accum_out=acc[:, NSLICE + g : NSLICE + g + 1],
                        )
                    # dd: always ACT
                    nc.scalar.activation(
                        a_scr[:, :cw],
                        ds,
                        mybir.ActivationFunctionType.Square,
                        accum_out=acc[:, 2 * NSLICE + g : 2 * NSLICE + g + 1],
                    )

            prev = None
            for kind, idx, w in _DMA_TILES:
                rtile = io.tile([P, w], f32, tag=f"r{kind}{idx if kind=='t' else ''}",
                                padded_shape=[P, FD] if kind == "f" else None,
                                bufs=3 if kind == "f" else 1)
                dtile = io.tile([P, w], f32, tag=f"d{kind}{idx if kind=='t' else ''}",
                                padded_shape=[P, FD] if kind == "f" else None,
                                bufs=3 if kind == "f" else 1)
                # Single SP ring: a lone ring already sustains the ~430 GB/s
                # per-core cap, and splitting across the ACT ring measurably
                # worsened paired-core HBM arbitration (every even core
                # dropped to ~300 GB/s mid-run).
                nc.sync.dma_start(out=rtile[:], in_=src("r", kind, idx))
                nc.sync.dma_start(out=dtile[:], in_=src("d", kind, idx))
                if prev is not None:
                    compute(prev)
                prev = (rtile, dtile, (kind, idx))
            compute(prev)
            nc.sync.dma_start(out=o[:], in_=acc[:])
    nc.compile()
    return nc


def _get_nc():
    if "nc" not in _CACHE:
        _CACHE["nc"] = _build()
    return _CACHE["nc"]


def _pack(arrays):
    flat = np.concatenate([np.asarray(a, dtype=np.float32).reshape(-1) for a in arrays])
    assert flat.size == TOTAL
    buf = np.zeros((C, PADDED_PER_CORE), dtype=np.float32)
    for c in range(C):
        buf[c, :PER_CORE] = flat[c * PER_CORE : (c + 1) * PER_CORE]
    nmain = NFULL * P * FD
    main = buf[:, :nmain].reshape(C, NFULL, P, FD)
    tails = []
    off = nmain
    for w in TAILS:
        tails.append(buf[:, off : off + P * w].reshape(C, P, w))
        off += P * w
    return main, tails


def _run(inputs, trace=False, trace_cores=None):
    rmain, rtails = _pack([inputs[k] for k in _REC_KEYS])
    dmain, dtails = _pack([inputs[k] for k in _DATA_KEYS])
    in_maps = []
    for c in range(C):
        m = {"r0": rmain[c], "d0": dmain[c]}
        for j in range(len(TAILS)):
            m[f"rt{j}"] = rtails[j][c]
            m[f"dt{j}"] = dtails[j][c]
        in_maps.append(m)
    kwargs = {}
    if trace_cores is not None:
        kwargs["trace_cores"] = trace_cores
    res = run_bass_kernel_spmd(
        _get_nc(), in_maps, core_ids=list(range(C)), trace=trace, **kwargs
    )
    tot = np.zeros(3, dtype=np.float64)
    for m in res.results:
        a = m["o"].astype(np.float64)
        tot[0] += a[:, :NSLICE].sum()
        tot[1] += a[:, NSLICE : 2 * NSLICE].sum()
        tot[2] += a[:, 2 * NSLICE :].sum()
    sp, rn, dn = tot
    out = 1.0 - sp / (np.sqrt(rn) * np.sqrt(dn))
    return np.array(out, dtype=np.float32), res


def kernel(**inputs):
    out, _ = _run(inputs, trace=False)
    return out


def kernel_traced(**inputs):
    out, res = _run(inputs, trace=True, trace_cores=list(range(C)))
    return out, res


# revision 6
# speedup vs baseline: 1.2430x; 1.1053x over previous
"""Trainium2 Bass kernel for fused multi-tensor cosine-similarity loss.

Computes 1 - <r,d> / (|r| |d|) over 10 gradient tensors (5 rec + 5 data,
45,675,264 f32 elements per side), data-parallel across 8 NeuronCores.

Strategy (memory-bound, ~45.7 MB HBM traffic per core, per-core DMA cap
~435 GB/s, paired cores share ~716 GB/s):
  - Host packs each side into a flat f32 stream, zero-padded to
    8 cores x 44,605 columns x 128 partitions; per core that is five
    contiguous [128, 8192] tiles plus 2048/1024/573-column tail chunks
    (descending so the final DMA's compute drain is minimal).
  - Dual-ring DMA: the rec stream is issued on the SP (sync) HWDGE ring,
    the data stream on the Activation ring.  Two concurrent request
    streams keep more DMAs outstanding, which wins a fair share of the
    paired-core HBM arbitration (single-ring cores can get starved to
    ~300 GB/s when their partner core streams at 420).
  - Few, large DMAs (8 per side vs 44 total in v1) cut the semaphore
    count, which directly shrinks the Tile epilogue (each allocated
    semaphore costs an individual clear instruction at exit).
  - Compute in [128, 2048] slices: DVE does the dot (r*d with row-sum
    accum) on every slice plus r^2 on odd slices; ACT does d^2 on every
    slice plus r^2 on even slices.  Engine scratch outs go to PSUM
    (4 banks each), freeing SBUF entirely for DMA double-buffering.
  - All per-slice partial sums land as columns of one [128, 69]
    accumulator, DMA'd out once; host reduces in float64 and applies
    the final cosine combine.
"""

import sys

import numpy as np

_REPO = "/opt/trn_rl_repo"
if _REPO not in sys.path:
    sys.path.insert(0, _REPO)

import concourse.bacc as bacc
import concourse.mybir as mybir
from concourse.bass_utils import run_bass_kernel_spmd
from concourse.tile import TileContext

C = 8  # cores
P = 128  # SBUF partitions
FD = 4096  # columns per full DMA tile (2 MiB)
FC = 2048  # columns per compute slice (PSUM-scratch sized)
TOTAL = 45_675_264  # elements per side (sum of the 5 tensor sizes)
PER_CORE = TOTAL // C  # 5,709,408
COLS = -(-PER_CORE // P)  # 44,605 columns per core (32 pad elems)
NFULL = COLS // FD  # 10 full [P, FD] tiles
TAILS = [2048, 1024, COLS - NFULL * FD - 3072]  # 2048 + 1024 + 573
assert NFULL * FD + sum(TAILS) == COLS
PADDED_PER_CORE = P * COLS  # 5,709,440

# DMA tile list: (kind, index_or_tag, ncols). Full tiles first, then
# descending tail chunks so the last-arriving data needs the least compute.
_DMA_TILES = [("f", i, FD) for i in range(NFULL)] + [
    ("t", j, w) for j, w in enumerate(TAILS)
]
# compute slices per DMA tile: list of (col_offset_in_tile, width, global_slice)
_SLICES = {}
_g = 0
for kind, idx, w in _DMA_TILES:
    sl = []
    off = 0
    while off < w:
        cw = min(FC, w - off)
        sl.append((off, cw, _g))
        _g += 1
        off += cw
    _SLICES[(kind, idx)] = sl
NSLICE = _g  # 23
ACC_W = 3 * NSLICE  # dot | rr | dd column blocks

_REC_KEYS = ("rec_emb", "rec_qkv", "rec_proj", "rec_fc1", "rec_fc2")
_DATA_KEYS = ("data_emb", "data_qkv", "data_proj", "data_fc1", "data_fc2")

_CACHE = {}


def _build():
    nc = bacc.Bacc("TRN2", target_bir_lowering=False, debug=False)
    f32 = mybir.dt.float32
    r0 = nc.declare_dram_parameter("r0", [NFULL, P, FD], f32, isOutput=False)
    d0 = nc.declare_dram_parameter("d0", [NFULL, P, FD], f32, isOutput=False)
    rt = [
        nc.declare_dram_parameter(f"rt{j}", [P, w], f32, isOutput=False)
        for j, w in enumerate(TAILS)
    ]
    dt = [
        nc.declare_dram_parameter(f"dt{j}", [P, w], f32, isOutput=False)
        for j, w in enumerate(TAILS)
    ]
    o = nc.declare_dram_parameter("o", [P, ACC_W], f32, isOutput=True)

    def src(side, kind, idx):
        if kind == "f":
            return (r0 if side == "r" else d0)[idx]
        return (rt if side == "r" else dt)[idx][:]

    with TileContext(nc) as tc:
        with (
            tc.tile_pool(name="io", bufs=2) as io,
            tc.tile_pool(name="accp", bufs=1) as accp,
            tc.psum_pool(name="scr", bufs=1) as scr,
        ):
            acc = accp.tile([P, ACC_W], f32)
            v_scr = scr.tile([P, FC], f32, tag="v")  # DVE scratch, 4 banks
            a_scr = scr.tile([P, FC], f32, tag="a")  # ACT scratch, 4 banks

            def compute(tiles):
                rtile, dtile, key = tiles
                for off, cw, g in _SLICES[key]:
                    rs = rtile[:, off : off + cw]
                    ds = dtile[:, off : off + cw]
                    # dot: out = (rs bypass 1.0) * ds; accum_out = row-sum
                    nc.vector.scalar_tensor_tensor(
                        out=v_scr[:, :cw],
                        in0=rs,
                        scalar=1.0,
                        in1=ds,
                        op0=mybir.AluOpType.bypass,
                        op1=mybir.AluOpType.mult,
                        accum_out=acc[:, g : g + 1],
                    )
                    # rr: alternate engines so DVE/ACT stay balanced
                    if g % 2 == 1:
                        nc.vector.scalar_tensor_tensor(
                            out=v_scr[:, :cw],
                            in0=rs,
                            scalar=1.0,
                            in1=rs,
                            op0=mybir.AluOpType.bypass,
                            op1=mybir.AluOpType.mult,
                            accum_out=acc[:, NSLICE + g : NSLICE + g + 1],
                        )
                    else:
                        nc.scalar.activation(
                            a_scr[:, :cw],
                            rs,
                            mybir.ActivationFunctionType.Square,
                            accum_out=acc[:, NSLICE + g : NSLICE + g + 1],
                        )
                    # dd: always ACT
                    nc.scalar.activation(
                        a_scr[:, :cw],
                        ds,
                        mybir.ActivationFunctionType.Square,
                        accum_out=acc[:, 2 * NSLICE + g : 2 * NSLICE + g + 1],
                    )

            prev = None
            for kind, idx, w in _DMA_TILES:
                rtile = io.tile([P, w], f32, tag=f"r{kind}{idx if kind=='t' else ''}",
                                padded_shape=[P, FD] if kind == "f" else None,
                                bufs=3 if kind == "f" else 1)
                dtile = io.tile([P, w], f32, tag=f"d{kind}{idx if kind=='t' else ''}",
                                padded_shape=[P, FD] if kind == "f" else None,
                                bufs=3 if kind == "f" else 1)
                # Single SP ring: a lone ring already sustains the ~430 GB/s
                # per-core cap, and splitting across the ACT ring measurably
                # worsened paired-core HBM arbitration (every even core
                # dropped to ~300 GB/s mid-run).
                nc.sync.dma_start(out=rtile[:], in_=src("r", kind, idx))
                nc.sync.dma_start(out=dtile[:], in_=src("d", kind, idx))
                if prev is not None:
                    compute(prev)
                prev = (rtile, dtile, (kind, idx))
            compute(prev)
            nc.sync.dma_start(out=o[:], in_=acc[:])
    nc.compile()
    return nc


def _get_nc():
    if "nc" not in _CACHE:
        _CACHE["nc"] = _build()
    return _CACHE["nc"]


def _pack(arrays):
    flat = np.concatenate([np.asarray(a, dtype=np.float32).reshape(-1) for a in arrays])
    assert flat.size == TOTAL
    buf = np.zeros((C, PADDED_PER_CORE), dtype=np.float32)
    for c in range(C):
        buf[c, :PER_CORE] = flat[c * PER_CORE : (c + 1) * PER_CORE]
    nmain = NFULL * P * FD
    main = buf[:, :nmain].reshape(C, NFULL, P, FD)
    tails = []
    off = nmain
    for w in TAILS:
        tails.append(buf[:, off : off + P * w].reshape(C, P, w))
        off += P * w
    return main, tails


def _run(inputs, trace=False, trace_cores=None):
    rmain, rtails = _pack([inputs[k] for k in _REC_KEYS])
    dmain, dtails = _pack([inputs[k] for k in _DATA_KEYS])
    in_maps = []
    for c in range(C):
        m = {"r0": rmain[c], "d0": dmain[c]}
        for j in range(len(TAILS)):
            m[f"rt{j}"] = rtails[j][c]
            m[f"dt{j}"] = dtails[j][c]
        in_maps.append(m)
    kwargs = {}
    if trace_cores is not None:
        kwargs["trace_cores"] = trace_cores
    res = run_bass_kernel_spmd(
        _get_nc(), in_maps, core_ids=list(range(C)), trace=trace, **kwargs
    )
    tot = np.zeros(3, dtype=np.float64)
    for m in res.results:
        a = m["o"].astype(np.float64)
        tot[0] += a[:, :NSLICE].sum()
        tot[1] += a[:, NSLICE : 2 * NSLICE].sum()
        tot[2] += a[:, 2 * NSLICE :].sum()
    sp, rn, dn = tot
    out = 1.0 - sp / (np.sqrt(rn) * np.sqrt(dn))
    return np.array(out, dtype=np.float32), res


def kernel(**inputs):
    out, _ = _run(inputs, trace=False)
    return out


def kernel_traced(_trace_cores=None, **inputs):
    out, res = _run(inputs, trace=True, trace_cores=_trace_cores)
    return out, res
